# revision 33
# baseline (speedup 1.0000x reference)
"""Multi-head self-attention + LayerNorm, sharded over 8 TRN2 NeuronCores.

Problem: x[4, 2048, 1024], 16 heads x 64 dim, causal attention, output
projection, LayerNorm.  Sharding: core c handles batch c//2 and head-group
c%2 (8 heads).  All 8 cores run one SPMD program; the output projection
produces partial sums which are pair-wise AllReduced (bf16) on device, then
each core applies the final LayerNorm.  Host gathers batch b from core 2*b.

Dtypes: projections / QK^T / output projection run in float32r (fp32
container, mantissa rounded to 11 explicit bits; full PE rate at N>=256).
The BIR verifier requires f32r matmul operands to be produced "rounded":
DRAM inputs are pre-rounded on the host and declared f32r; on-chip operands
are produced by ACT/DVE ops with f32r output dtype.  The AV matmul
(softmax weights x V) runs in bf16, as does the whole post-out-proj y path
(partial sums, AllReduce, LayerNorm, output) -- LN renormalizes, so bf16
rounding of y costs ~0.4% against a 2e-2 budget.

Pipeline: one pass over 512-row t-tiles.  Attention for tile tt hosts the
QKV projection of tile tt+1 as "filler" matmul units popped one per k-tile
iteration, keeping the PE dense so the HAM clock-gate stays at 2.4 GHz.
LayerNorm of tile tt-1 is emitted after out_proj(tt), a full attention
phase after its AllReduce was triggered, so its loads never stall a FIFO
queue that attention work is behind.  rstd is exp(-0.5*ln(var+eps)) --
both functions live in one ACT table set (loads rewritten post-pass to the
combined natural_log_exp_and_others set), so the kernel does exactly one
~2.7us table load.
"""

import numpy as np

import concourse.bass as bass
import concourse.mybir as mybir
import concourse.tile as tile
from concourse import bacc, library_config
from concourse.bass_utils import run_bass_kernel_spmd

# Problem constants (hardcoded per harness contract)
B, T, C = 4, 2048, 1024
H, D = 16, 64
HG = 2                 # head groups (cores per batch)
HPG = H // HG          # heads per group = 8
CG = C // HG           # channels per group = 512
SCALE = D ** -0.5      # 0.125
LN_EPS = 1e-5

QT = 512               # q tile (moving free dim)
KT = 128               # k tile (PE contraction tile)
NQT = T // QT          # 4
NKC = T // KT          # 16
NIC = C // 128         # 8 input-channel chunks
NDC = CG // 128        # 4 output d-chunks per group

F32 = mybir.dt.float32
F32R = mybir.dt.float32r
BF16 = mybir.dt.bfloat16

REPLICA_GROUPS = [[0, 1], [2, 3], [4, 5], [6, 7]]


def build_program():
    """Build + compile the single-core SPMD Bass program. Returns (nc, io)."""
    nc = bacc.Bacc(
        "TRN2",
        target_bir_lowering=False,
        debug=False,
        enable_asserts=False,
        num_devices=8,
    )

    # ---- DRAM I/O ----  (f32r inputs are pre-rounded fp32 on the host)
    xT = nc.dram_tensor("xT", [C, T], F32R, kind="ExternalInput")
    wqT = nc.dram_tensor("wqT", [C, CG], F32R, kind="ExternalInput")
    wkT = nc.dram_tensor("wkT", [C, CG], F32R, kind="ExternalInput")
    wvT = nc.dram_tensor("wvT", [C, CG], F32R, kind="ExternalInput")
    wpT = nc.dram_tensor("wpT", [CG, C], BF16, kind="ExternalInput")
    gamma = nc.dram_tensor("gamma", [C], BF16, kind="ExternalInput")
    beta = nc.dram_tensor("beta", [C], BF16, kind="ExternalInput")
    # single causal triangle for the 128x128 diagonal sub-block:
    # mask[k, q] = 1.0 where k <= q
    mask = nc.dram_tensor("mask", [KT, KT], BF16, kind="ExternalInput")
    y_out = nc.dram_tensor("y", [T, C], BF16, kind="ExternalOutput")

    with tile.TileContext(nc) as tc:
        _body(tc, xT, wqT, wkT, wvT, wpT, gamma, beta, mask, y_out)

    # Rewrite every ACT table load to the one set containing all our
    # functions (exp, ln, copy) and dedupe: 1 load instead of a swap per
    # exp<->ln transition.  compile()'s internal pass then inserts nothing.
    nc.insert_act_table_loads()
    from concourse.hw_specs import get_activation_tables
    tabs = list(get_activation_tables(nc.m.arch).keys())
    combined = tabs.index("natural_log_exp_and_others")
    for blk in nc.main_func.blocks:
        kept = False
        newinsts = []
        for i in blk.instructions:
            if isinstance(i, mybir.InstLoadActFuncSet):
                i.act_func_set_id = combined
                if kept:
                    continue
                kept = True
            newinsts.append(i)
        blk.instructions = newinsts

    nc.compile()
    io = dict(inputs=["xT", "wqT", "wkT", "wvT", "wpT", "gamma", "beta", "mask"],
              output="y")
    return nc, io


def _body(tc, xT, wqT, wkT, wvT, wpT, gamma, beta, mask, y_out):
    nc = tc.nc

    # ---------- persistent SBUF ----------
    persist = tc.alloc_tile_pool(name="persist", bufs=1)
    # K^T in [128 part, d-chunk, t] layout; head h lives at partition rows
    # 64*(h%2) .. +64 of chunk h//2.
    kT_sb = persist.tile([128, NDC, T], BF16)
    # V in [t(128-chunks) part, k-chunk, head, 65] layout; col 64 is the ones
    # column providing the softmax denominator in the AV matmul.
    v_sb = persist.tile([128, NKC, HPG, 65], BF16)
    mask_sb = persist.tile([128, KT], BF16)
    eps_sb = persist.tile([128, 1], F32)
    wq_sb = persist.tile([128, NIC, CG], F32R)
    wk_sb = persist.tile([128, NIC, CG], F32R)
    wv_sb = persist.tile([128, NIC, CG], F32R)
    wp_sb = persist.tile([128, NDC, C], BF16)
    gamma_sb = persist.tile([128, C], BF16)
    beta_sb = persist.tile([128, C], BF16)

    nc.vector.memset(eps_sb, LN_EPS)
    # f32r/bf16 matmul operands cannot be memset directly; round via a copy
    ones_f = persist.tile([128, 128], F32)
    ones_sb = persist.tile([65, 64], F32R)
    nc.vector.memset(ones_f, 1.0)
    nc.scalar.copy(ones_sb, ones_f[0:65, 0:64])
    # ones columns of V
    nc.scalar.copy(
        v_sb[:, :, :, 64],
        ones_f[:, 0:NKC * HPG].rearrange("p (a b) -> p a b", a=NKC),
    )

    with (
        tc.tile_pool(name="qrot", bufs=3) as qpool,
        tc.tile_pool(name="xstream", bufs=2) as xpool,
        tc.tile_pool(name="psP", bufs=2, space="PSUM") as psP,
        tc.tile_pool(name="psS", bufs=2, space="PSUM") as psS,
        tc.tile_pool(name="psO", bufs=2, space="PSUM") as psO,
        tc.tile_pool(name="pT", bufs=3) as ppool,
        tc.tile_pool(name="recipp", bufs=2) as rpool,
        tc.tile_pool(name="bcastp", bufs=1) as bpool,
        tc.tile_pool(name="onormp", bufs=2) as opool,
        tc.tile_pool(name="ytile", bufs=4) as ypool,
        tc.tile_pool(name="lnld", bufs=5) as lnldpool,
        tc.tile_pool(name="lnst", bufs=3) as lnpool,
        tc.tile_pool(name="lnout", bufs=2) as lnopool,
        tc.tile_pool(name="dram", bufs=1, space="DRAM") as dram,
    ):
        y_parts = [dram.tile([QT, C], BF16, name=f"y_part{i}") for i in range(NQT)]
        y_reds = [dram.tile([QT, C], BF16, name=f"y_red{i}") for i in range(NQT)]

        # ---- prologue DMAs, spread over 4 queues so startup isn't
        # serialized on one descriptor ring ----
        def load_x(tt, eng):
            x_t = xpool.tile([128, NIC, QT], F32R, name="x_t")
            t0 = tt * QT
            xv = xT.ap()[:, t0:t0 + QT].rearrange("(a p) t -> p a t", p=128)
            for ic in range(NIC):
                eng.dma_start(out=x_t[:, ic, :], in_=xv[:, ic, :])
            return x_t

        nc.sync.dma_start(out=mask_sb, in_=mask.ap())
        wkv = wkT.ap().rearrange("(a p) o -> p a o", p=128)
        for ic in range(NIC):
            nc.sync.dma_start(out=wk_sb[:, ic, :], in_=wkv[:, ic, :])
        x_tiles = {0: load_x(0, nc.scalar)}
        nc.gpsimd.dma_start(out=wq_sb, in_=wqT.ap().rearrange("(a p) o -> p a o", p=128))
        nc.scalar.dma_start(out=wv_sb, in_=wvT.ap().rearrange("(a p) o -> p a o", p=128))
        nc.scalar.dma_start(out=wp_sb, in_=wpT.ap().rearrange("(a p) o -> p a o", p=128))
        nc.gpsimd.dma_start(out=gamma_sb, in_=gamma.ap().unsqueeze(0).to_broadcast([128, C]))
        nc.gpsimd.dma_start(out=beta_sb, in_=beta.ap().unsqueeze(0).to_broadcast([128, C]))
        x_tiles[1] = load_x(1, nc.gpsimd)

        # ---- projection of tile tt, as lists of filler units ----
        # Each unit is ~8 matmuls (N=512) + one PSUM evacuation; units are
        # popped into attention k-loops.  K/Q units must finish before
        # attention of tile tt starts; V units are only needed when tile
        # tt's own k-tiles reach the AV stage (the last 4 k-iterations of
        # each pair), so they can slide into attention(tt) itself -- this
        # keeps the PE denser in the late, ACT-bound tiles.
        def project_units(tt, x_t, qT_t):
            units = []

            def kq_unit(w_sb, dst, dc, evac_eng):
                def run():
                    ps = psP.tile([128, QT], F32, tag="ps", name="ps_p")
                    for ic in range(NIC):
                        nc.tensor.matmul(
                            ps,
                            w_sb[:, ic, dc * 128:(dc + 1) * 128],
                            x_t[:, ic, :],
                            start=(ic == 0), stop=(ic == NIC - 1),
                        )
                    out_ap = (qT_t[:, dc, :] if dst is None
                              else dst[:, dc, tt * QT:(tt + 1) * QT])
                    if evac_eng == "scalar":
                        nc.scalar.copy(out_ap, ps)
                    else:
                        nc.vector.tensor_copy(out_ap, ps)
                return run

            def v_unit(j):
                def run():
                    kc = tt * (QT // 128) + j
                    ps = psP.tile([128, CG], F32, tag="ps", name="ps_v")
                    for ic in range(NIC):
                        nc.tensor.matmul(
                            ps,
                            x_t[:, ic, j * 128:(j + 1) * 128],
                            wv_sb[:, ic, :],
                            start=(ic == 0), stop=(ic == NIC - 1),
                        )
                    nc.vector.tensor_copy(
                        v_sb[:, kc, :, 0:64],
                        ps.rearrange("p (h d) -> p h d", h=HPG),
                    )
                return run

            kunits = [kq_unit(wk_sb, kT_sb, dc,
                              "scalar" if dc % 2 == 0 else "vector")
                      for dc in range(NDC)]
            qunits = [kq_unit(wq_sb, None, dc,
                              "scalar" if dc % 2 == 1 else "vector")
                      for dc in range(NDC)]
            vunits = [v_unit(j) for j in range(QT // 128)]
            return kunits, qunits, vunits

        def trim(kt, nkt):
            # first valid q column of this k-tile within the q-tile
            diag = kt - (nkt - 4)
            return (0 if diag < 0 else diag * KT), diag

        def s_pair(hp, qT_t, kt, nkt):
            # S^T[k, q] = K[k, :] . Q[q, :]; heads 2hp / 2hp+1 live on
            # partition rows 0-63 / 64-127 -> row-tiled, run concurrent
            off, _ = trim(kt, nkt)
            ps_s = psS.tile([128, 2, QT], F32, tag="pss", name="ps_s")
            for e in range(2):
                r0, r1 = 64 * e, 64 * e + 64
                nc.tensor.matmul(
                    ps_s[:, e, off:],
                    kT_sb[r0:r1, hp, kt * KT:(kt + 1) * KT],
                    qT_t[r0:r1, hp, off:],
                    start=True, stop=True,
                )
            return ps_s

        def exp_mask(ps_s, kt, nkt):
            off, diag = trim(kt, nkt)
            p_t = ppool.tile([128, 2, QT], BF16, tag="pt", name="p_t")
            # exp over both heads' tiles in one ACT call (trimmed)
            nc.scalar.activation(
                p_t[:, :, off:], ps_s[:, :, off:],
                mybir.ActivationFunctionType.Exp, scale=SCALE,
            )
            if diag >= 0:
                # only the 128-col diagonal window needs masking; columns
                # right of it are fully inside the causal region
                for e in range(2):
                    nc.vector.tensor_mul(
                        p_t[:, e, off:off + KT], p_t[:, e, off:off + KT],
                        mask_sb,
                    )
            return p_t

        def attention(hp, qt, qT_t, preheat, next_ctx, filler):
            # next_ctx = (hp', qt', qT_t') of the following pair, or None.
            # Returns the (ps_s, p_t) preheat for that pair, emitted before
            # this pair's normalize so the next pair's first AV never stalls.
            nkt = (qt + 1) * (QT // KT)  # causal k-extent in 128-tiles

            ps_o = [psO.tile([65, QT], F32, tag="pso", name=f"ps_o{e}")
                    for e in range(2)]

            if preheat is None:
                ps_s_cur = s_pair(hp, qT_t, 0, nkt)
                pt_cur = None
            else:
                ps_s_cur, pt_cur = preheat
            for kt in range(nkt):
                ps_s_next = s_pair(hp, qT_t, kt + 1, nkt) if kt + 1 < nkt else None
                off, diag = trim(kt, nkt)
                p_t = pt_cur if pt_cur is not None else exp_mask(ps_s_cur, kt, nkt)
                pt_cur = None
                for e in range(2):
                    h = 2 * hp + e
                    nc.tensor.matmul(
                        ps_o[e][:, off:],
                        v_sb[:, kt, h, :],
                        p_t[:, e, off:],
                        start=(kt == 0), stop=(kt == nkt - 1),
                    )
                ps_s_cur = ps_s_next
                if filler:
                    filler.pop(0)()

            preheat_next = None
            if next_ctx is not None:
                hp2, qt2, qT_t2 = next_ctx
                nkt2 = (qt2 + 1) * (QT // KT)
                ps0 = s_pair(hp2, qT_t2, 0, nkt2)
                preheat_next = (ps0, exp_mask(ps0, 0, nkt2))

            # normalize: O^T[d, q] /= denom[q]; write into qT_t (=out^T).
            # denom row -> f32r, broadcast across 64 partitions via a ones
            # outer-product on the PE, approx-reciprocal, then one DVE
            # multiply per head.
            d_r = rpool.tile([65, 2, QT], F32R, tag="recip")
            rb = bpool.tile([64, 2, QT], F32, tag="rbcast")
            for e in range(2):
                nc.vector.tensor_copy(d_r[64:65, e, :], ps_o[e][64:65, :])
                db_ps = psP.tile([64, QT], F32, tag="ps", name="db_ps")
                nc.tensor.matmul(
                    db_ps, ones_sb[64:65, :], d_r[64:65, e, :],
                    start=True, stop=True,
                )
                nc.vector.reciprocal_approx_fast(out=rb[:, e, :], in_=db_ps)
            for e in range(2):
                if e == 0:
                    nc.vector.tensor_mul(
                        qT_t[0:64, hp, :], ps_o[e][0:64, :], rb[:, e, :])
                else:
                    o_n = opool.tile([64, QT], BF16, tag="onorm")
                    nc.vector.tensor_mul(o_n, ps_o[e][0:64, :], rb[:, e, :])
                    nc.sync.dma_start(out=qT_t[64:128, hp, :], in_=o_n)
            return preheat_next

        def out_proj_units(qt, qT_t, evac="vector"):
            # y_part rows [512*qt, 512*qt+512) = out^T.T @ WpT  (partial
            # sums, bf16); one unit per (row chunk, 512-col chunk)
            y_part = y_parts[qt]
            units = []

            def unit(i, ct):
                def run():
                    ps = psP.tile([128, QT], F32, tag="ps", name="ps_y")
                    for cc in range(NDC):
                        nc.tensor.matmul(
                            ps,
                            qT_t[:, cc, i * 128:(i + 1) * 128],
                            wp_sb[:, cc, ct * QT:(ct + 1) * QT],
                            start=(cc == 0), stop=(cc == NDC - 1),
                        )
                    y_sb = ypool.tile([128, QT], BF16, tag="ysb", name="y_sb")
                    if evac == "vector":
                        nc.vector.tensor_copy(y_sb, ps)
                    else:
                        nc.scalar.copy(y_sb, ps)
                    nc.gpsimd.dma_start(
                        out=y_part[i * 128:(i + 1) * 128,
                                   ct * QT:(ct + 1) * QT],
                        in_=y_sb)
                return run

            for i in range(QT // 128):
                for ct in range(C // QT):
                    units.append(unit(i, ct))
            return units

        def ar_unit(qt, r0=0, r1=QT):
            def run():
                nc.gpsimd.collective_compute(
                    "AllReduce",
                    mybir.AluOpType.add,
                    replica_groups=REPLICA_GROUPS,
                    ins=[y_parts[qt][r0:r1, :]],
                    outs=[y_reds[qt][r0:r1, :]],
                )
            return run

        def layer_norm(qt, i0=0, ntn=QT // 128):
            # normalize `ntn` 128-row blocks of reduced tile qt starting at
            # block i0: single bf16 load per block, stats + resident
            # normalize, bf16 output.
            y_red = y_reds[qt]
            y_hs = []
            mv_all = lnpool.tile([128, ntn, 2], F32, tag="mv")
            for i in range(ntn):
                y_h = lnldpool.tile([128, C], BF16, tag="yh", name="y_h")
                nc.gpsimd.dma_start(
                    out=y_h, in_=y_red[(i0 + i) * 128:(i0 + i + 1) * 128, :])
                y_hs.append(y_h)
                stats = lnpool.tile([128, 2, 6], F32, tag="stats")
                for s in range(2):
                    nc.vector.bn_stats(out=stats[:, s, :],
                                       in_=y_h[:, s * QT:(s + 1) * QT])
                nc.vector.bn_aggr(out=mv_all[:, i, :], in_=stats)
            # rstd = exp(-0.5 * ln(var + eps)): both funcs in one ACT set,
            # so no table swaps against the attention exps.
            rstd = lnpool.tile([128, ntn], F32, tag="rstd")
            nc.scalar.activation(
                out=rstd, in_=mv_all[:, :, 1],
                func=mybir.ActivationFunctionType.Ln,
                bias=eps_sb, scale=1.0,
            )
            nc.scalar.activation(
                out=rstd, in_=rstd,
                func=mybir.ActivationFunctionType.Exp, scale=-0.5,
            )
            for i in range(ntn):
                tn = qt * (QT // 128) + i0 + i
                y_o = lnopool.tile([128, C], BF16, tag="yo", name="y_o")
                nc.vector.scalar_tensor_tensor(
                    out=y_o, in0=y_hs[i],
                    scalar=mv_all[:, i, 0:1], in1=gamma_sb,
                    op0=mybir.AluOpType.subtract,
                    op1=mybir.AluOpType.mult,
                )
                nc.vector.scalar_tensor_tensor(
                    out=y_o, in0=y_o,
                    scalar=rstd[:, i:i + 1], in1=beta_sb,
                    op0=mybir.AluOpType.mult,
                    op1=mybir.AluOpType.add,
                )
                nc.gpsimd.dma_start(
                    out=y_out.ap()[tn * 128:(tn + 1) * 128, :], in_=y_o)

        # ---- the fused pipeline over 512-row t-tiles ----
        # Attention for tile tt hosts, as PE filler units popped one per
        # k-tile: out_proj of tile tt-1 (then its AllReduce trigger), and
        # the QKV projection of tile tt+1.  LayerNorm of tile tt-1 runs
        # right after tile tt's pairs -- its AllReduce completed mid-tile.
        qT_tiles = {}

        def get_qT(tt):
            if tt not in qT_tiles:
                qT_tiles[tt] = qpool.tile([128, NDC, QT], BF16, name="qT_t")
            return qT_tiles[tt]

        # project tile 0 inline (its fillers have no host loop yet); K(0)
        # and V(0) are needed immediately (tile 0's k-tiles are all
        # diagonal).  For later tiles, Q(tt) units run as fillers in tile
        # tt-1 (attention tt needs all of Q up front), while K(tt)/V(tt)
        # defer into tile tt itself (only needed at its diagonal k-tiles)
        # -- this shifts PE filler work toward the late, ACT-bound tiles.
        k0, q0, v0 = project_units(0, x_tiles[0], get_qT(0))
        for u in k0 + q0 + v0:
            u()

        kv_next = {}
        ph = None
        for tt in range(NQT):
            filler = []
            if tt in kv_next:
                kk, vv = kv_next.pop(tt)
                filler += [kk[0]] + vv + kk[1:]
            if tt > 0:
                filler += out_proj_units(tt - 1, get_qT(tt - 1))
                filler.append(ar_unit(tt - 1))
            if tt + 1 < NQT:
                kn, qn, vn = project_units(tt + 1, x_tiles[tt + 1], get_qT(tt + 1))
                filler += qn
                kv_next[tt + 1] = (kn, vn)
            for hp in range(HPG // 2):
                # prefetch x two tiles ahead, emitted mid-attention so the
                # buffer-reuse wait on the trigger is already satisfied and
                # never parks the queue (parking delays the cc stream)
                if hp == 2 and tt + 2 < NQT:
                    x_tiles[tt + 2] = load_x(tt + 2, nc.sync)
                if hp + 1 < HPG // 2:
                    next_ctx = (hp + 1, tt, get_qT(tt))
                elif tt + 1 < NQT:
                    next_ctx = (0, tt + 1, get_qT(tt + 1))
                else:
                    next_ctx = None
                ph = attention(hp, tt, get_qT(tt), ph, next_ctx, filler)
            for u in filler:
                u()
            if tt > 0:
                layer_norm(tt - 1)

        # tail: out_proj(3) with the AllReduce split in halves so LN of the
        # first half overlaps the second half's collective
        last = NQT - 1
        op3 = out_proj_units(last, get_qT(last), evac="scalar")
        for u in op3[0:4]:
            u()
        ar_unit(last, 0, QT // 2)()
        for u in op3[4:8]:
            u()
        ar_unit(last, QT // 2, QT)()
        layer_norm(last, 0, 2)
        layer_norm(last, 2, 2)

    persist.release()


_PROG = None


def _get_program():
    global _PROG
    if _PROG is None:
        _PROG = build_program()
    return _PROG


def _round_f32r(a):
    """Round fp32 to the f32r grid (11 explicit mantissa bits, RNE-ish)."""
    bits = np.ascontiguousarray(a, np.float32).view(np.uint32)
    r = ((bits.astype(np.uint64) + 0x800) & 0xFFFFF000).astype(np.uint32)
    return r.view(np.float32)


def make_in_maps(x, Wk, Wq, Wv, Wp, gamma, beta):
    import ml_dtypes
    x = np.asarray(x, dtype=np.float32)
    k = np.arange(KT)[:, None]
    q = np.arange(KT)[None, :]
    mask = (k <= q).astype(np.float32).astype(ml_dtypes.bfloat16)
    in_maps = []
    for c in range(8):
        b, hg = c // HG, c % HG
        sl = slice(hg * CG, (hg + 1) * CG)
        in_maps.append({
            "xT": _round_f32r(x[b].T),
            "wqT": _round_f32r(np.asarray(Wq, np.float32)[sl, :].T),
            "wkT": _round_f32r(np.asarray(Wk, np.float32)[sl, :].T),
            "wvT": _round_f32r(np.asarray(Wv, np.float32)[sl, :].T),
            "wpT": np.asarray(Wp, np.float32)[:, sl].T.astype(ml_dtypes.bfloat16),
            "gamma": np.asarray(gamma, np.float32).astype(ml_dtypes.bfloat16),
            "beta": np.asarray(beta, np.float32).astype(ml_dtypes.bfloat16),
            "mask": mask,
        })
    return in_maps


def kernel(x, Wk, Wq, Wv, Wp, gamma, beta, _trace=False, _trace_kwargs=None):
    nc, io = _get_program()
    in_maps = make_in_maps(x, Wk, Wq, Wv, Wp, gamma, beta)
    res = run_bass_kernel_spmd(
        nc, in_maps, core_ids=list(range(8)),
        trace=_trace, **(_trace_kwargs or {}),
    )
    out = np.stack([np.asarray(res.results[HG * b]["y"], dtype=np.float32)
                    for b in range(B)])
    if _trace:
        kernel.last_results = res
    return out


# revision 34
# speedup vs baseline: 1.1182x; 1.1182x over previous
"""Multi-head self-attention + LayerNorm, sharded over 8 TRN2 NeuronCores.

Problem: x[4, 2048, 1024], 16 heads x 64 dim, causal attention, output
projection, LayerNorm.  Sharding: core c handles batch c//2 and head-group
c%2 (8 heads).  All 8 cores run one SPMD program; the output projection
produces partial sums which are pair-wise AllReduced (bf16) on device, then
each core applies the final LayerNorm.  Host gathers batch b from core 2*b.

Dtypes: projections / QK^T / output projection run in float32r (fp32
container, mantissa rounded to 11 explicit bits; full PE rate at N>=256).
The BIR verifier requires f32r matmul operands to be produced "rounded":
DRAM inputs are pre-rounded on the host and declared f32r; on-chip operands
are produced by ACT/DVE ops with f32r output dtype.  The AV matmul
(softmax weights x V) runs in bf16, as does the whole post-out-proj y path
(partial sums, AllReduce, LayerNorm, output) -- LN renormalizes, so bf16
rounding of y costs ~0.4% against a 2e-2 budget.

Pipeline: one pass over 512-row t-tiles.  Attention for tile tt hosts the
QKV projection of tile tt+1 as "filler" matmul units popped one per k-tile
iteration, keeping the PE dense so the HAM clock-gate stays at 2.4 GHz.
LayerNorm of tile tt-1 is emitted after out_proj(tt), a full attention
phase after its AllReduce was triggered, so its loads never stall a FIFO
queue that attention work is behind.  rstd is exp(-0.5*ln(var+eps)) --
both functions live in one ACT table set (loads rewritten post-pass to the
combined natural_log_exp_and_others set), so the kernel does exactly one
~2.7us table load.
"""

import numpy as np

import concourse.bass as bass
import concourse.mybir as mybir
import concourse.tile as tile
from concourse import bacc, library_config
from concourse.bass_utils import run_bass_kernel_spmd

# Problem constants (hardcoded per harness contract)
B, T, C = 4, 2048, 1024
H, D = 16, 64
HG = 2                 # head groups (cores per batch)
HPG = H // HG          # heads per group = 8
CG = C // HG           # channels per group = 512
SCALE = D ** -0.5      # 0.125
LN_EPS = 1e-5

QT = 512               # q tile (moving free dim)
KT = 128               # k tile (PE contraction tile)
NQT = T // QT          # 4
NKC = T // KT          # 16
NIC = C // 128         # 8 input-channel chunks
NDC = CG // 128        # 4 output d-chunks per group

F32 = mybir.dt.float32
F32R = mybir.dt.float32r
BF16 = mybir.dt.bfloat16

REPLICA_GROUPS = [[0, 1], [2, 3], [4, 5], [6, 7]]


def build_program():
    """Build + compile the single-core SPMD Bass program. Returns (nc, io)."""
    nc = bacc.Bacc(
        "TRN2",
        target_bir_lowering=False,
        debug=False,
        enable_asserts=False,
        num_devices=8,
    )

    # ---- DRAM I/O ----  (f32r inputs are pre-rounded fp32 on the host)
    xT = nc.dram_tensor("xT", [C, T], F32R, kind="ExternalInput")
    wqT = nc.dram_tensor("wqT", [C, CG], F32R, kind="ExternalInput")
    wkT = nc.dram_tensor("wkT", [C, CG], F32R, kind="ExternalInput")
    wvT = nc.dram_tensor("wvT", [C, CG], F32R, kind="ExternalInput")
    wpT = nc.dram_tensor("wpT", [CG, C], BF16, kind="ExternalInput")
    gamma = nc.dram_tensor("gamma", [C], BF16, kind="ExternalInput")
    beta = nc.dram_tensor("beta", [C], BF16, kind="ExternalInput")
    # single causal triangle for the 128x128 diagonal sub-block:
    # mask[k, q] = 1.0 where k <= q
    mask = nc.dram_tensor("mask", [KT, KT], BF16, kind="ExternalInput")
    y_out = nc.dram_tensor("y", [T, C], BF16, kind="ExternalOutput")

    with tile.TileContext(nc) as tc:
        _body(tc, xT, wqT, wkT, wvT, wpT, gamma, beta, mask, y_out)

    # Rewrite every ACT table load to the one set containing all our
    # functions (exp, ln, copy) and dedupe: 1 load instead of a swap per
    # exp<->ln transition.  compile()'s internal pass then inserts nothing.
    nc.insert_act_table_loads()
    from concourse.hw_specs import get_activation_tables
    tabs = list(get_activation_tables(nc.m.arch).keys())
    combined = tabs.index("natural_log_exp_and_others")
    for blk in nc.main_func.blocks:
        kept = False
        newinsts = []
        for i in blk.instructions:
            if isinstance(i, mybir.InstLoadActFuncSet):
                i.act_func_set_id = combined
                if kept:
                    continue
                kept = True
            newinsts.append(i)
        blk.instructions = newinsts

    nc.compile()
    io = dict(inputs=["xT", "wqT", "wkT", "wvT", "wpT", "gamma", "beta", "mask"],
              output="y")
    return nc, io


def _body(tc, xT, wqT, wkT, wvT, wpT, gamma, beta, mask, y_out):
    nc = tc.nc

    # ---------- persistent SBUF ----------
    persist = tc.alloc_tile_pool(name="persist", bufs=1)
    # K^T in [128 part, d-chunk, t] layout; head h lives at partition rows
    # 64*(h%2) .. +64 of chunk h//2.
    kT_sb = persist.tile([128, NDC, T], BF16)
    # V in [t(128-chunks) part, k-chunk, head, 65] layout; col 64 is the ones
    # column providing the softmax denominator in the AV matmul.
    v_sb = persist.tile([128, NKC, HPG, 65], BF16)
    mask_sb = persist.tile([128, KT], BF16)
    eps_sb = persist.tile([128, 1], F32)
    wq_sb = persist.tile([128, NIC, CG], F32R)
    wk_sb = persist.tile([128, NIC, CG], F32R)
    wv_sb = persist.tile([128, NIC, CG], F32R)
    wp_sb = persist.tile([128, NDC, C], BF16)
    gamma_sb = persist.tile([128, C], BF16)
    beta_sb = persist.tile([128, C], BF16)

    nc.vector.memset(eps_sb, LN_EPS)
    # f32r/bf16 matmul operands cannot be memset directly; round via a copy
    ones_f = persist.tile([128, 128], F32)
    ones_sb = persist.tile([65, 64], F32R)
    nc.vector.memset(ones_f, 1.0)
    nc.scalar.copy(ones_sb, ones_f[0:65, 0:64])
    # ones columns of V
    nc.scalar.copy(
        v_sb[:, :, :, 64],
        ones_f[:, 0:NKC * HPG].rearrange("p (a b) -> p a b", a=NKC),
    )

    with (
        tc.tile_pool(name="qrot", bufs=3) as qpool,
        tc.tile_pool(name="xstream", bufs=2) as xpool,
        tc.tile_pool(name="psP", bufs=2, space="PSUM") as psP,
        tc.tile_pool(name="psS", bufs=2, space="PSUM") as psS,
        tc.tile_pool(name="psO", bufs=2, space="PSUM") as psO,
        tc.tile_pool(name="pT", bufs=3) as ppool,
        tc.tile_pool(name="recipp", bufs=2) as rpool,
        tc.tile_pool(name="bcastp", bufs=1) as bpool,
        tc.tile_pool(name="onormp", bufs=2) as opool,
        tc.tile_pool(name="ytile", bufs=4) as ypool,
        tc.tile_pool(name="lnld", bufs=5) as lnldpool,
        tc.tile_pool(name="lnst", bufs=3) as lnpool,
        tc.tile_pool(name="lnout", bufs=2) as lnopool,
        tc.tile_pool(name="dram", bufs=1, space="DRAM") as dram,
    ):
        y_parts = [dram.tile([QT, C], BF16, name=f"y_part{i}") for i in range(NQT)]
        y_reds = [dram.tile([QT, C], BF16, name=f"y_red{i}") for i in range(NQT)]

        # ---- prologue DMAs, spread over 4 queues so startup isn't
        # serialized on one descriptor ring ----
        def load_x(tt, eng):
            x_t = xpool.tile([128, NIC, QT], F32R, name="x_t")
            t0 = tt * QT
            xv = xT.ap()[:, t0:t0 + QT].rearrange("(a p) t -> p a t", p=128)
            for ic in range(NIC):
                eng.dma_start(out=x_t[:, ic, :], in_=xv[:, ic, :])
            return x_t

        nc.sync.dma_start(out=mask_sb, in_=mask.ap())
        wkv = wkT.ap().rearrange("(a p) o -> p a o", p=128)
        for ic in range(NIC):
            nc.sync.dma_start(out=wk_sb[:, ic, :], in_=wkv[:, ic, :])
        x_tiles = {0: load_x(0, nc.scalar)}
        nc.gpsimd.dma_start(out=wq_sb, in_=wqT.ap().rearrange("(a p) o -> p a o", p=128))
        nc.scalar.dma_start(out=wv_sb, in_=wvT.ap().rearrange("(a p) o -> p a o", p=128))
        nc.scalar.dma_start(out=wp_sb, in_=wpT.ap().rearrange("(a p) o -> p a o", p=128))
        nc.gpsimd.dma_start(out=gamma_sb, in_=gamma.ap().unsqueeze(0).to_broadcast([128, C]))
        nc.gpsimd.dma_start(out=beta_sb, in_=beta.ap().unsqueeze(0).to_broadcast([128, C]))
        x_tiles[1] = load_x(1, nc.gpsimd)

        # ---- projection of tile tt, as lists of filler units ----
        # Each unit is ~8 matmuls (N=512) + one PSUM evacuation; units are
        # popped into attention k-loops.  K/Q units must finish before
        # attention of tile tt starts; V units are only needed when tile
        # tt's own k-tiles reach the AV stage (the last 4 k-iterations of
        # each pair), so they can slide into attention(tt) itself -- this
        # keeps the PE denser in the late, ACT-bound tiles.
        def project_units(tt, x_t, qT_t):
            units = []

            def kq_unit(w_sb, dst, dc, evac_eng):
                def run():
                    ps = psP.tile([128, QT], F32, tag="ps", name="ps_p")
                    for ic in range(NIC):
                        nc.tensor.matmul(
                            ps,
                            w_sb[:, ic, dc * 128:(dc + 1) * 128],
                            x_t[:, ic, :],
                            start=(ic == 0), stop=(ic == NIC - 1),
                        )
                    out_ap = (qT_t[:, dc, :] if dst is None
                              else dst[:, dc, tt * QT:(tt + 1) * QT])
                    if evac_eng == "scalar":
                        nc.scalar.copy(out_ap, ps)
                    else:
                        nc.vector.tensor_copy(out_ap, ps)
                return run

            def v_unit(j):
                def run():
                    kc = tt * (QT // 128) + j
                    ps = psP.tile([128, CG], F32, tag="ps", name="ps_v")
                    for ic in range(NIC):
                        nc.tensor.matmul(
                            ps,
                            x_t[:, ic, j * 128:(j + 1) * 128],
                            wv_sb[:, ic, :],
                            start=(ic == 0), stop=(ic == NIC - 1),
                        )
                    nc.vector.tensor_copy(
                        v_sb[:, kc, :, 0:64],
                        ps.rearrange("p (h d) -> p h d", h=HPG),
                    )
                return run

            kunits = [kq_unit(wk_sb, kT_sb, dc,
                              "scalar" if dc % 2 == 0 else "vector")
                      for dc in range(NDC)]
            qunits = [kq_unit(wq_sb, None, dc,
                              "scalar" if dc % 2 == 1 else "vector")
                      for dc in range(NDC)]
            vunits = [v_unit(j) for j in range(QT // 128)]
            return kunits, qunits, vunits

        def trim(kt, nkt):
            # first valid q column of this k-tile within the q-tile
            diag = kt - (nkt - 4)
            return (0 if diag < 0 else diag * KT), diag

        def s_pair(hp, qT_t, kt, nkt):
            # S^T[k, q] = K[k, :] . Q[q, :]; heads 2hp / 2hp+1 live on
            # partition rows 0-63 / 64-127 -> row-tiled, run concurrent
            off, _ = trim(kt, nkt)
            ps_s = psS.tile([128, 2, QT], F32, tag="pss", name="ps_s")
            for e in range(2):
                r0, r1 = 64 * e, 64 * e + 64
                nc.tensor.matmul(
                    ps_s[:, e, off:],
                    kT_sb[r0:r1, hp, kt * KT:(kt + 1) * KT],
                    qT_t[r0:r1, hp, off:],
                    start=True, stop=True,
                )
            return ps_s

        def exp_mask(ps_s, kt, nkt):
            off, diag = trim(kt, nkt)
            p_t = ppool.tile([128, 2, QT], BF16, tag="pt", name="p_t")
            # exp over both heads' tiles in one ACT call (trimmed)
            nc.scalar.activation(
                p_t[:, :, off:], ps_s[:, :, off:],
                mybir.ActivationFunctionType.Exp, scale=SCALE,
            )
            if diag >= 0:
                # only the 128-col diagonal window needs masking; columns
                # right of it are fully inside the causal region
                for e in range(2):
                    nc.vector.tensor_mul(
                        p_t[:, e, off:off + KT], p_t[:, e, off:off + KT],
                        mask_sb,
                    )
            return p_t

        def attention(hp, qt, qT_t, preheat, next_ctx, filler):
            # next_ctx = (hp', qt', qT_t') of the following pair, or None.
            # Returns the (ps_s, p_t) preheat for that pair, emitted before
            # this pair's normalize so the next pair's first AV never stalls.
            nkt = (qt + 1) * (QT // KT)  # causal k-extent in 128-tiles

            ps_o = [psO.tile([65, QT], F32, tag="pso", name=f"ps_o{e}")
                    for e in range(2)]

            if preheat is None:
                ps_s_cur = s_pair(hp, qT_t, 0, nkt)
                pt_cur = None
            else:
                ps_s_cur, pt_cur = preheat
            for kt in range(nkt):
                ps_s_next = s_pair(hp, qT_t, kt + 1, nkt) if kt + 1 < nkt else None
                off, diag = trim(kt, nkt)
                p_t = pt_cur if pt_cur is not None else exp_mask(ps_s_cur, kt, nkt)
                pt_cur = None
                for e in range(2):
                    h = 2 * hp + e
                    nc.tensor.matmul(
                        ps_o[e][:, off:],
                        v_sb[:, kt, h, :],
                        p_t[:, e, off:],
                        start=(kt == 0), stop=(kt == nkt - 1),
                    )
                ps_s_cur = ps_s_next
                if filler:
                    filler.pop(0)()

            preheat_next = None
            if next_ctx is not None:
                hp2, qt2, qT_t2 = next_ctx
                nkt2 = (qt2 + 1) * (QT // KT)
                ps0 = s_pair(hp2, qT_t2, 0, nkt2)
                preheat_next = (ps0, exp_mask(ps0, 0, nkt2))

            # normalize: O^T[d, q] /= denom[q]; write into qT_t (=out^T).
            # denom row -> f32r, broadcast across 64 partitions via a ones
            # outer-product on the PE, approx-reciprocal, then one DVE
            # multiply per head.
            d_r = rpool.tile([65, 2, QT], F32R, tag="recip")
            rb = bpool.tile([64, 2, QT], F32, tag="rbcast")
            for e in range(2):
                nc.vector.tensor_copy(d_r[64:65, e, :], ps_o[e][64:65, :])
                db_ps = psP.tile([64, QT], F32, tag="ps", name="db_ps")
                nc.tensor.matmul(
                    db_ps, ones_sb[64:65, :], d_r[64:65, e, :],
                    start=True, stop=True,
                )
                nc.vector.reciprocal_approx_fast(out=rb[:, e, :], in_=db_ps)
            for e in range(2):
                if e == 0:
                    nc.vector.tensor_mul(
                        qT_t[0:64, hp, :], ps_o[e][0:64, :], rb[:, e, :])
                else:
                    o_n = opool.tile([64, QT], BF16, tag="onorm")
                    nc.vector.tensor_mul(o_n, ps_o[e][0:64, :], rb[:, e, :])
                    nc.sync.dma_start(out=qT_t[64:128, hp, :], in_=o_n)
            return preheat_next

        def out_proj_units(qt, qT_t, evac="vector"):
            # y_part rows [512*qt, 512*qt+512) = out^T.T @ WpT  (partial
            # sums, bf16); one unit per (row chunk, 512-col chunk)
            y_part = y_parts[qt]
            units = []

            def unit(i, ct):
                def run():
                    ps = psP.tile([128, QT], F32, tag="ps", name="ps_y")
                    for cc in range(NDC):
                        nc.tensor.matmul(
                            ps,
                            qT_t[:, cc, i * 128:(i + 1) * 128],
                            wp_sb[:, cc, ct * QT:(ct + 1) * QT],
                            start=(cc == 0), stop=(cc == NDC - 1),
                        )
                    y_sb = ypool.tile([128, QT], BF16, tag="ysb", name="y_sb")
                    if evac == "vector":
                        nc.vector.tensor_copy(y_sb, ps)
                    else:
                        nc.scalar.copy(y_sb, ps)
                    nc.gpsimd.dma_start(
                        out=y_part[i * 128:(i + 1) * 128,
                                   ct * QT:(ct + 1) * QT],
                        in_=y_sb)
                return run

            for i in range(QT // 128):
                for ct in range(C // QT):
                    units.append(unit(i, ct))
            return units

        def ar_unit(qt, r0=0, r1=QT):
            def run():
                nc.gpsimd.collective_compute(
                    "AllReduce",
                    mybir.AluOpType.add,
                    replica_groups=REPLICA_GROUPS,
                    ins=[y_parts[qt][r0:r1, :]],
                    outs=[y_reds[qt][r0:r1, :]],
                )
            return run

        def layer_norm(qt, i0=0, ntn=QT // 128):
            # normalize `ntn` 128-row blocks of reduced tile qt starting at
            # block i0: single bf16 load per block, stats + resident
            # normalize, bf16 output.
            y_red = y_reds[qt]
            y_hs = []
            mv_all = lnpool.tile([128, ntn, 2], F32, tag="mv")
            for i in range(ntn):
                y_h = lnldpool.tile([128, C], BF16, tag="yh", name="y_h")
                nc.gpsimd.dma_start(
                    out=y_h, in_=y_red[(i0 + i) * 128:(i0 + i + 1) * 128, :])
                y_hs.append(y_h)
                stats = lnpool.tile([128, 2, 6], F32, tag="stats")
                for s in range(2):
                    nc.vector.bn_stats(out=stats[:, s, :],
                                       in_=y_h[:, s * QT:(s + 1) * QT])
                nc.vector.bn_aggr(out=mv_all[:, i, :], in_=stats)
            # rstd = exp(-0.5 * ln(var + eps)): both funcs in one ACT set,
            # so no table swaps against the attention exps.
            rstd = lnpool.tile([128, ntn], F32, tag="rstd")
            nc.scalar.activation(
                out=rstd, in_=mv_all[:, :, 1],
                func=mybir.ActivationFunctionType.Ln,
                bias=eps_sb, scale=1.0,
            )
            nc.scalar.activation(
                out=rstd, in_=rstd,
                func=mybir.ActivationFunctionType.Exp, scale=-0.5,
            )
            for i in range(ntn):
                tn = qt * (QT // 128) + i0 + i
                y_o = lnopool.tile([128, C], BF16, tag="yo", name="y_o")
                nc.vector.scalar_tensor_tensor(
                    out=y_o, in0=y_hs[i],
                    scalar=mv_all[:, i, 0:1], in1=gamma_sb,
                    op0=mybir.AluOpType.subtract,
                    op1=mybir.AluOpType.mult,
                )
                nc.vector.scalar_tensor_tensor(
                    out=y_o, in0=y_o,
                    scalar=rstd[:, i:i + 1], in1=beta_sb,
                    op0=mybir.AluOpType.mult,
                    op1=mybir.AluOpType.add,
                )
                nc.gpsimd.dma_start(
                    out=y_out.ap()[tn * 128:(tn + 1) * 128, :], in_=y_o)

        # ---- the fused pipeline over 512-row t-tiles ----
        # Attention for tile tt hosts, as PE filler units popped one per
        # k-tile: out_proj of tile tt-1 (then its AllReduce trigger), and
        # the QKV projection of tile tt+1.  LayerNorm of tile tt-1 runs
        # right after tile tt's pairs -- its AllReduce completed mid-tile.
        qT_tiles = {}

        def get_qT(tt):
            if tt not in qT_tiles:
                qT_tiles[tt] = qpool.tile([128, NDC, QT], BF16, name="qT_t")
            return qT_tiles[tt]

        # project tile 0 inline (its fillers have no host loop yet); K(0)
        # and V(0) are needed immediately (tile 0's k-tiles are all
        # diagonal).  For later tiles, Q(tt) units run as fillers in tile
        # tt-1 (attention tt needs all of Q up front), while K(tt)/V(tt)
        # defer into tile tt itself (only needed at its diagonal k-tiles)
        # -- this shifts PE filler work toward the late, ACT-bound tiles.
        k0, q0, v0 = project_units(0, x_tiles[0], get_qT(0))
        for u in k0 + q0 + v0:
            u()

        kv_next = {}
        ph = None
        for tt in range(NQT):
            filler = []
            if tt in kv_next:
                kk, vv = kv_next.pop(tt)
                filler += [kk[0]] + vv + kk[1:]
            if tt > 0:
                filler += out_proj_units(tt - 1, get_qT(tt - 1))
                filler.append(ar_unit(tt - 1))
            if tt + 1 < NQT:
                kn, qn, vn = project_units(tt + 1, x_tiles[tt + 1], get_qT(tt + 1))
                filler += qn
                kv_next[tt + 1] = (kn, vn)
            for hp in range(HPG // 2):
                # prefetch x two tiles ahead, emitted mid-attention so the
                # buffer-reuse wait on the trigger is already satisfied and
                # never parks the queue (parking delays the cc stream)
                if hp == 2 and tt + 2 < NQT:
                    x_tiles[tt + 2] = load_x(tt + 2, nc.sync)
                if hp + 1 < HPG // 2:
                    next_ctx = (hp + 1, tt, get_qT(tt))
                elif tt + 1 < NQT:
                    next_ctx = (0, tt + 1, get_qT(tt + 1))
                else:
                    next_ctx = None
                ph = attention(hp, tt, get_qT(tt), ph, next_ctx, filler)
            for u in filler:
                u()
            if tt > 0:
                layer_norm(tt - 1)

        # tail: out_proj(3), one AllReduce (splitting it pays the ~15us
        # collective fixed cost twice, serialized -- measured worse), then
        # LN(3)
        last = NQT - 1
        for u in out_proj_units(last, get_qT(last), evac="scalar"):
            u()
        ar_unit(last)()
        layer_norm(last)

    persist.release()


_PROG = None


def _get_program():
    global _PROG
    if _PROG is None:
        _PROG = build_program()
    return _PROG


def _round_f32r(a):
    """Round fp32 to the f32r grid (11 explicit mantissa bits, RNE-ish)."""
    bits = np.ascontiguousarray(a, np.float32).view(np.uint32)
    r = ((bits.astype(np.uint64) + 0x800) & 0xFFFFF000).astype(np.uint32)
    return r.view(np.float32)


def make_in_maps(x, Wk, Wq, Wv, Wp, gamma, beta):
    import ml_dtypes
    x = np.asarray(x, dtype=np.float32)
    k = np.arange(KT)[:, None]
    q = np.arange(KT)[None, :]
    mask = (k <= q).astype(np.float32).astype(ml_dtypes.bfloat16)
    in_maps = []
    for c in range(8):
        b, hg = c // HG, c % HG
        sl = slice(hg * CG, (hg + 1) * CG)
        in_maps.append({
            "xT": _round_f32r(x[b].T),
            "wqT": _round_f32r(np.asarray(Wq, np.float32)[sl, :].T),
            "wkT": _round_f32r(np.asarray(Wk, np.float32)[sl, :].T),
            "wvT": _round_f32r(np.asarray(Wv, np.float32)[sl, :].T),
            "wpT": np.asarray(Wp, np.float32)[:, sl].T.astype(ml_dtypes.bfloat16),
            "gamma": np.asarray(gamma, np.float32).astype(ml_dtypes.bfloat16),
            "beta": np.asarray(beta, np.float32).astype(ml_dtypes.bfloat16),
            "mask": mask,
        })
    return in_maps


def kernel(x, Wk, Wq, Wv, Wp, gamma, beta, _trace=False, _trace_kwargs=None):
    nc, io = _get_program()
    in_maps = make_in_maps(x, Wk, Wq, Wv, Wp, gamma, beta)
    res = run_bass_kernel_spmd(
        nc, in_maps, core_ids=list(range(8)),
        trace=_trace, **(_trace_kwargs or {}),
    )
    out = np.stack([np.asarray(res.results[HG * b]["y"], dtype=np.float32)
                    for b in range(B)])
    if _trace:
        kernel.last_results = res
    return out


# revision 37
# speedup vs baseline: 1.1526x; 1.0308x over previous
"""Multi-head self-attention + LayerNorm, sharded over 8 TRN2 NeuronCores.

Problem: x[4, 2048, 1024], 16 heads x 64 dim, causal attention, output
projection, LayerNorm.  Sharding: core c handles batch c//2 and head-group
c%2 (8 heads).  All 8 cores run one SPMD program; the output projection
produces partial sums which are pair-wise AllReduced (bf16) on device, then
each core applies the final LayerNorm.  Host gathers batch b from core 2*b.

Dtypes: projections / QK^T / output projection run in float32r (fp32
container, mantissa rounded to 11 explicit bits; full PE rate at N>=256).
The BIR verifier requires f32r matmul operands to be produced "rounded":
DRAM inputs are pre-rounded on the host and declared f32r; on-chip operands
are produced by ACT/DVE ops with f32r output dtype.  The AV matmul
(softmax weights x V) runs in bf16, as does the whole post-out-proj y path
(partial sums, AllReduce, LayerNorm, output) -- LN renormalizes, so bf16
rounding of y costs ~0.4% against a 2e-2 budget.

Pipeline: one pass over 512-row t-tiles.  Attention for tile tt hosts the
QKV projection of tile tt+1 as "filler" matmul units popped one per k-tile
iteration, keeping the PE dense so the HAM clock-gate stays at 2.4 GHz.
LayerNorm of tile tt-1 is emitted after out_proj(tt), a full attention
phase after its AllReduce was triggered, so its loads never stall a FIFO
queue that attention work is behind.  rstd is exp(-0.5*ln(var+eps)) --
both functions live in one ACT table set (loads rewritten post-pass to the
combined natural_log_exp_and_others set), so the kernel does exactly one
~2.7us table load.
"""

import numpy as np

import concourse.bass as bass
import concourse.mybir as mybir
import concourse.tile as tile
from concourse import bacc, library_config
from concourse.bass_utils import run_bass_kernel_spmd

# Problem constants (hardcoded per harness contract)
B, T, C = 4, 2048, 1024
H, D = 16, 64
HG = 2                 # head groups (cores per batch)
HPG = H // HG          # heads per group = 8
CG = C // HG           # channels per group = 512
SCALE = D ** -0.5      # 0.125
LN_EPS = 1e-5

QT = 512               # q tile (moving free dim)
KT = 128               # k tile (PE contraction tile)
NQT = T // QT          # 4
NKC = T // KT          # 16
NIC = C // 128         # 8 input-channel chunks
NDC = CG // 128        # 4 output d-chunks per group

F32 = mybir.dt.float32
F32R = mybir.dt.float32r
BF16 = mybir.dt.bfloat16

REPLICA_GROUPS = [[0, 1], [2, 3], [4, 5], [6, 7]]


def build_program():
    """Build + compile the single-core SPMD Bass program. Returns (nc, io)."""
    nc = bacc.Bacc(
        "TRN2",
        target_bir_lowering=False,
        debug=False,
        enable_asserts=False,
        num_devices=8,
    )

    # ---- DRAM I/O ----  (f32r inputs are pre-rounded fp32 on the host)
    xT = nc.dram_tensor("xT", [C, T], F32R, kind="ExternalInput")
    wqT = nc.dram_tensor("wqT", [C, CG], F32R, kind="ExternalInput")
    wkT = nc.dram_tensor("wkT", [C, CG], F32R, kind="ExternalInput")
    wvT = nc.dram_tensor("wvT", [C, CG], F32R, kind="ExternalInput")
    wpT = nc.dram_tensor("wpT", [CG, C], BF16, kind="ExternalInput")
    gamma = nc.dram_tensor("gamma", [C], BF16, kind="ExternalInput")
    beta = nc.dram_tensor("beta", [C], BF16, kind="ExternalInput")
    # single causal triangle for the 128x128 diagonal sub-block:
    # mask[k, q] = 1.0 where k <= q
    mask = nc.dram_tensor("mask", [KT, KT], BF16, kind="ExternalInput")
    y_out = nc.dram_tensor("y", [T, C], BF16, kind="ExternalOutput")

    with tile.TileContext(nc) as tc:
        _body(tc, xT, wqT, wkT, wvT, wpT, gamma, beta, mask, y_out)

    # Rewrite every ACT table load to the one set containing all our
    # functions (exp, ln, copy) and dedupe: 1 load instead of a swap per
    # exp<->ln transition.  compile()'s internal pass then inserts nothing.
    nc.insert_act_table_loads()
    from concourse.hw_specs import get_activation_tables
    tabs = list(get_activation_tables(nc.m.arch).keys())
    combined = tabs.index("natural_log_exp_and_others")
    for blk in nc.main_func.blocks:
        kept = False
        newinsts = []
        for i in blk.instructions:
            if isinstance(i, mybir.InstLoadActFuncSet):
                i.act_func_set_id = combined
                if kept:
                    continue
                kept = True
            newinsts.append(i)
        blk.instructions = newinsts

    nc.compile()
    io = dict(inputs=["xT", "wqT", "wkT", "wvT", "wpT", "gamma", "beta", "mask"],
              output="y")
    return nc, io


def _body(tc, xT, wqT, wkT, wvT, wpT, gamma, beta, mask, y_out):
    nc = tc.nc

    # ---------- persistent SBUF ----------
    persist = tc.alloc_tile_pool(name="persist", bufs=1)
    # K^T in [128 part, d-chunk, t] layout; head h lives at partition rows
    # 64*(h%2) .. +64 of chunk h//2.
    kT_sb = persist.tile([128, NDC, T], BF16)
    # V in [t(128-chunks) part, k-chunk, head, 65] layout; col 64 is the ones
    # column providing the softmax denominator in the AV matmul.
    v_sb = persist.tile([128, NKC, HPG, 65], BF16)
    mask_sb = persist.tile([128, KT], BF16)
    eps_sb = persist.tile([128, 1], F32)
    wq_sb = persist.tile([128, NIC, CG], F32R)
    wk_sb = persist.tile([128, NIC, CG], F32R)
    wv_sb = persist.tile([128, NIC, CG], F32R)
    wp_sb = persist.tile([128, NDC, C], BF16)
    gamma_sb = persist.tile([128, C], BF16)
    beta_sb = persist.tile([128, C], BF16)

    nc.vector.memset(eps_sb, LN_EPS)
    # f32r/bf16 matmul operands cannot be memset directly; round via a copy
    ones_f = persist.tile([128, 128], F32)
    ones_sb = persist.tile([65, 64], F32R)
    nc.vector.memset(ones_f, 1.0)
    nc.scalar.copy(ones_sb, ones_f[0:65, 0:64])
    # ones columns of V
    nc.scalar.copy(
        v_sb[:, :, :, 64],
        ones_f[:, 0:NKC * HPG].rearrange("p (a b) -> p a b", a=NKC),
    )

    with (
        tc.tile_pool(name="qrot", bufs=3) as qpool,
        tc.tile_pool(name="xstream", bufs=2) as xpool,
        tc.tile_pool(name="psP", bufs=2, space="PSUM") as psP,
        tc.tile_pool(name="psS", bufs=2, space="PSUM") as psS,
        tc.tile_pool(name="psO", bufs=2, space="PSUM") as psO,
        tc.tile_pool(name="pT", bufs=3) as ppool,
        tc.tile_pool(name="recipp", bufs=2) as rpool,
        tc.tile_pool(name="bcastp", bufs=1) as bpool,
        tc.tile_pool(name="onormp", bufs=2) as opool,
        tc.tile_pool(name="ytile", bufs=4) as ypool,
        tc.tile_pool(name="lnld", bufs=5) as lnldpool,
        tc.tile_pool(name="lnst", bufs=3) as lnpool,
        tc.tile_pool(name="lnout", bufs=2) as lnopool,
        tc.tile_pool(name="dram", bufs=1, space="DRAM") as dram,
    ):
        y_parts = [dram.tile([QT, C], BF16, name=f"y_part{i}") for i in range(NQT)]
        y_reds = [dram.tile([QT, C], BF16, name=f"y_red{i}") for i in range(NQT)]

        # ---- prologue DMAs, spread over 4 queues so startup isn't
        # serialized on one descriptor ring ----
        def load_x(tt, eng):
            x_t = xpool.tile([128, NIC, QT], F32R, name="x_t")
            t0 = tt * QT
            xv = xT.ap()[:, t0:t0 + QT].rearrange("(a p) t -> p a t", p=128)
            for ic in range(NIC):
                eng.dma_start(out=x_t[:, ic, :], in_=xv[:, ic, :])
            return x_t

        # Tiny warm-up collective, triggered first on the gpsimd queue: it
        # absorbs the ~11us first-trigger delay of the cc stream and forces
        # the cross-core rendezvous to happen during startup DMAs instead of
        # skewing into the first real AllReduce.
        cc_warm_i = dram.tile([64], BF16, name="cc_warm_i")
        cc_warm_o = dram.tile([64], BF16, name="cc_warm_o")
        nc.gpsimd.collective_compute(
            "AllReduce", mybir.AluOpType.add,
            replica_groups=REPLICA_GROUPS,
            ins=[cc_warm_i[:]], outs=[cc_warm_o[:]],
        )
        nc.sync.dma_start(out=mask_sb, in_=mask.ap())
        wkv = wkT.ap().rearrange("(a p) o -> p a o", p=128)
        for ic in range(NIC):
            nc.sync.dma_start(out=wk_sb[:, ic, :], in_=wkv[:, ic, :])
        x_tiles = {0: load_x(0, nc.scalar)}
        nc.gpsimd.dma_start(out=wq_sb, in_=wqT.ap().rearrange("(a p) o -> p a o", p=128))
        nc.scalar.dma_start(out=wv_sb, in_=wvT.ap().rearrange("(a p) o -> p a o", p=128))
        nc.scalar.dma_start(out=wp_sb, in_=wpT.ap().rearrange("(a p) o -> p a o", p=128))
        nc.gpsimd.dma_start(out=gamma_sb, in_=gamma.ap().unsqueeze(0).to_broadcast([128, C]))
        nc.gpsimd.dma_start(out=beta_sb, in_=beta.ap().unsqueeze(0).to_broadcast([128, C]))
        x_tiles[1] = load_x(1, nc.gpsimd)

        # ---- projection of tile tt, as lists of filler units ----
        # Each unit is ~8 matmuls (N=512) + one PSUM evacuation; units are
        # popped into attention k-loops.  K/Q units must finish before
        # attention of tile tt starts; V units are only needed when tile
        # tt's own k-tiles reach the AV stage (the last 4 k-iterations of
        # each pair), so they can slide into attention(tt) itself -- this
        # keeps the PE denser in the late, ACT-bound tiles.
        def project_units(tt, x_t, qT_t):
            units = []

            def kq_unit(w_sb, dst, dc, evac_eng):
                def run():
                    ps = psP.tile([128, QT], F32, tag="ps", name="ps_p")
                    for ic in range(NIC):
                        nc.tensor.matmul(
                            ps,
                            w_sb[:, ic, dc * 128:(dc + 1) * 128],
                            x_t[:, ic, :],
                            start=(ic == 0), stop=(ic == NIC - 1),
                        )
                    out_ap = (qT_t[:, dc, :] if dst is None
                              else dst[:, dc, tt * QT:(tt + 1) * QT])
                    if evac_eng == "scalar":
                        nc.scalar.copy(out_ap, ps)
                    else:
                        nc.vector.tensor_copy(out_ap, ps)
                return run

            def v_unit(j):
                def run():
                    kc = tt * (QT // 128) + j
                    ps = psP.tile([128, CG], F32, tag="ps", name="ps_v")
                    for ic in range(NIC):
                        nc.tensor.matmul(
                            ps,
                            x_t[:, ic, j * 128:(j + 1) * 128],
                            wv_sb[:, ic, :],
                            start=(ic == 0), stop=(ic == NIC - 1),
                        )
                    nc.vector.tensor_copy(
                        v_sb[:, kc, :, 0:64],
                        ps.rearrange("p (h d) -> p h d", h=HPG),
                    )
                return run

            kunits = [kq_unit(wk_sb, kT_sb, dc,
                              "scalar" if dc % 2 == 0 else "vector")
                      for dc in range(NDC)]
            qunits = [kq_unit(wq_sb, None, dc,
                              "scalar" if dc % 2 == 1 else "vector")
                      for dc in range(NDC)]
            vunits = [v_unit(j) for j in range(QT // 128)]
            return kunits, qunits, vunits

        def trim(kt, nkt):
            # first valid q column of this k-tile within the q-tile
            diag = kt - (nkt - 4)
            return (0 if diag < 0 else diag * KT), diag

        def s_pair(hp, qT_t, kt, nkt):
            # S^T[k, q] = K[k, :] . Q[q, :]; heads 2hp / 2hp+1 live on
            # partition rows 0-63 / 64-127 -> row-tiled, run concurrent
            off, _ = trim(kt, nkt)
            ps_s = psS.tile([128, 2, QT], F32, tag="pss", name="ps_s")
            for e in range(2):
                r0, r1 = 64 * e, 64 * e + 64
                nc.tensor.matmul(
                    ps_s[:, e, off:],
                    kT_sb[r0:r1, hp, kt * KT:(kt + 1) * KT],
                    qT_t[r0:r1, hp, off:],
                    start=True, stop=True,
                )
            return ps_s

        def exp_mask(ps_s, kt, nkt):
            off, diag = trim(kt, nkt)
            p_t = ppool.tile([128, 2, QT], BF16, tag="pt", name="p_t")
            # exp over both heads' tiles in one ACT call (trimmed)
            nc.scalar.activation(
                p_t[:, :, off:], ps_s[:, :, off:],
                mybir.ActivationFunctionType.Exp, scale=SCALE,
            )
            if diag >= 0:
                # only the 128-col diagonal window needs masking; columns
                # right of it are fully inside the causal region
                for e in range(2):
                    nc.vector.tensor_mul(
                        p_t[:, e, off:off + KT], p_t[:, e, off:off + KT],
                        mask_sb,
                    )
            return p_t

        def attention(hp, qt, qT_t, preheat, next_ctx, filler):
            # next_ctx = (hp', qt', qT_t') of the following pair, or None.
            # Returns the (ps_s, p_t) preheat for that pair, emitted before
            # this pair's normalize so the next pair's first AV never stalls.
            nkt = (qt + 1) * (QT // KT)  # causal k-extent in 128-tiles

            ps_o = [psO.tile([65, QT], F32, tag="pso", name=f"ps_o{e}")
                    for e in range(2)]

            if preheat is None:
                ps_s_cur = s_pair(hp, qT_t, 0, nkt)
                pt_cur = None
            else:
                ps_s_cur, pt_cur = preheat
            for kt in range(nkt):
                ps_s_next = s_pair(hp, qT_t, kt + 1, nkt) if kt + 1 < nkt else None
                off, diag = trim(kt, nkt)
                p_t = pt_cur if pt_cur is not None else exp_mask(ps_s_cur, kt, nkt)
                pt_cur = None
                for e in range(2):
                    h = 2 * hp + e
                    nc.tensor.matmul(
                        ps_o[e][:, off:],
                        v_sb[:, kt, h, :],
                        p_t[:, e, off:],
                        start=(kt == 0), stop=(kt == nkt - 1),
                    )
                ps_s_cur = ps_s_next
                if filler:
                    filler.pop(0)()

            preheat_next = None
            if next_ctx is not None:
                hp2, qt2, qT_t2 = next_ctx
                nkt2 = (qt2 + 1) * (QT // KT)
                ps0 = s_pair(hp2, qT_t2, 0, nkt2)
                preheat_next = (ps0, exp_mask(ps0, 0, nkt2))

            # normalize: O^T[d, q] /= denom[q]; write into qT_t (=out^T).
            # denom row -> f32r, broadcast across 64 partitions via a ones
            # outer-product on the PE, approx-reciprocal, then one DVE
            # multiply per head.
            d_r = rpool.tile([65, 2, QT], F32R, tag="recip")
            rb = bpool.tile([64, 2, QT], F32, tag="rbcast")
            for e in range(2):
                nc.vector.tensor_copy(d_r[64:65, e, :], ps_o[e][64:65, :])
                db_ps = psP.tile([64, QT], F32, tag="ps", name="db_ps")
                nc.tensor.matmul(
                    db_ps, ones_sb[64:65, :], d_r[64:65, e, :],
                    start=True, stop=True,
                )
                nc.vector.reciprocal_approx_fast(out=rb[:, e, :], in_=db_ps)
            for e in range(2):
                if e == 0:
                    nc.vector.tensor_mul(
                        qT_t[0:64, hp, :], ps_o[e][0:64, :], rb[:, e, :])
                else:
                    o_n = opool.tile([64, QT], BF16, tag="onorm")
                    nc.vector.tensor_mul(o_n, ps_o[e][0:64, :], rb[:, e, :])
                    nc.sync.dma_start(out=qT_t[64:128, hp, :], in_=o_n)
            return preheat_next

        def out_proj_units(qt, qT_t, evac="vector"):
            # y_part rows [512*qt, 512*qt+512) = out^T.T @ WpT  (partial
            # sums, bf16); one unit per (row chunk, 512-col chunk)
            y_part = y_parts[qt]
            units = []

            def unit(i, ct):
                def run():
                    ps = psP.tile([128, QT], F32, tag="ps", name="ps_y")
                    for cc in range(NDC):
                        nc.tensor.matmul(
                            ps,
                            qT_t[:, cc, i * 128:(i + 1) * 128],
                            wp_sb[:, cc, ct * QT:(ct + 1) * QT],
                            start=(cc == 0), stop=(cc == NDC - 1),
                        )
                    y_sb = ypool.tile([128, QT], BF16, tag="ysb", name="y_sb")
                    if evac == "vector":
                        nc.vector.tensor_copy(y_sb, ps)
                    else:
                        nc.scalar.copy(y_sb, ps)
                    nc.gpsimd.dma_start(
                        out=y_part[i * 128:(i + 1) * 128,
                                   ct * QT:(ct + 1) * QT],
                        in_=y_sb)
                return run

            for i in range(QT // 128):
                for ct in range(C // QT):
                    units.append(unit(i, ct))
            return units

        def ar_unit(qt, r0=0, r1=QT):
            def run():
                nc.gpsimd.collective_compute(
                    "AllReduce",
                    mybir.AluOpType.add,
                    replica_groups=REPLICA_GROUPS,
                    ins=[y_parts[qt][r0:r1, :]],
                    outs=[y_reds[qt][r0:r1, :]],
                )
            return run

        def layer_norm(qt, i0=0, ntn=QT // 128):
            # normalize `ntn` 128-row blocks of reduced tile qt starting at
            # block i0: single bf16 load per block, stats + resident
            # normalize, bf16 output.
            y_red = y_reds[qt]
            y_hs = []
            mv_all = lnpool.tile([128, ntn, 2], F32, tag="mv")
            for i in range(ntn):
                y_h = lnldpool.tile([128, C], BF16, tag="yh", name="y_h")
                nc.gpsimd.dma_start(
                    out=y_h, in_=y_red[(i0 + i) * 128:(i0 + i + 1) * 128, :])
                y_hs.append(y_h)
                stats = lnpool.tile([128, 2, 6], F32, tag="stats")
                for s in range(2):
                    nc.vector.bn_stats(out=stats[:, s, :],
                                       in_=y_h[:, s * QT:(s + 1) * QT])
                nc.vector.bn_aggr(out=mv_all[:, i, :], in_=stats)
            # rstd = exp(-0.5 * ln(var + eps)): both funcs in one ACT set,
            # so no table swaps against the attention exps.
            rstd = lnpool.tile([128, ntn], F32, tag="rstd")
            nc.scalar.activation(
                out=rstd, in_=mv_all[:, :, 1],
                func=mybir.ActivationFunctionType.Ln,
                bias=eps_sb, scale=1.0,
            )
            nc.scalar.activation(
                out=rstd, in_=rstd,
                func=mybir.ActivationFunctionType.Exp, scale=-0.5,
            )
            for i in range(ntn):
                tn = qt * (QT // 128) + i0 + i
                y_o = lnopool.tile([128, C], BF16, tag="yo", name="y_o")
                nc.vector.scalar_tensor_tensor(
                    out=y_o, in0=y_hs[i],
                    scalar=mv_all[:, i, 0:1], in1=gamma_sb,
                    op0=mybir.AluOpType.subtract,
                    op1=mybir.AluOpType.mult,
                )
                nc.vector.scalar_tensor_tensor(
                    out=y_o, in0=y_o,
                    scalar=rstd[:, i:i + 1], in1=beta_sb,
                    op0=mybir.AluOpType.mult,
                    op1=mybir.AluOpType.add,
                )
                nc.gpsimd.dma_start(
                    out=y_out.ap()[tn * 128:(tn + 1) * 128, :], in_=y_o)

        # ---- the fused pipeline over 512-row t-tiles ----
        # Attention for tile tt hosts, as PE filler units popped one per
        # k-tile: out_proj of tile tt-1 (then its AllReduce trigger), and
        # the QKV projection of tile tt+1.  LayerNorm of tile tt-1 runs
        # right after tile tt's pairs -- its AllReduce completed mid-tile.
        qT_tiles = {}

        def get_qT(tt):
            if tt not in qT_tiles:
                qT_tiles[tt] = qpool.tile([128, NDC, QT], BF16, name="qT_t")
            return qT_tiles[tt]

        # project tile 0 inline (its fillers have no host loop yet); K(0)
        # and V(0) are needed immediately (tile 0's k-tiles are all
        # diagonal).  For later tiles, Q(tt) units run as fillers in tile
        # tt-1 (attention tt needs all of Q up front), while K(tt)/V(tt)
        # defer into tile tt itself (only needed at its diagonal k-tiles)
        # -- this shifts PE filler work toward the late, ACT-bound tiles.
        k0, q0, v0 = project_units(0, x_tiles[0], get_qT(0))
        for u in k0 + q0 + v0:
            u()

        kv_next = {}
        ph = None
        for tt in range(NQT):
            filler = []
            if tt in kv_next:
                kk, vv = kv_next.pop(tt)
                filler += [kk[0]] + vv + kk[1:]
            if tt > 0:
                filler += out_proj_units(tt - 1, get_qT(tt - 1))
                filler.append(ar_unit(tt - 1))
            if tt + 1 < NQT:
                kn, qn, vn = project_units(tt + 1, x_tiles[tt + 1], get_qT(tt + 1))
                filler += qn
                kv_next[tt + 1] = (kn, vn)
            for hp in range(HPG // 2):
                # prefetch x two tiles ahead, emitted mid-attention so the
                # buffer-reuse wait on the trigger is already satisfied and
                # never parks the queue (parking delays the cc stream)
                if hp == 2 and tt + 2 < NQT:
                    x_tiles[tt + 2] = load_x(tt + 2, nc.sync)
                if hp + 1 < HPG // 2:
                    next_ctx = (hp + 1, tt, get_qT(tt))
                elif tt + 1 < NQT:
                    next_ctx = (0, tt + 1, get_qT(tt + 1))
                else:
                    next_ctx = None
                ph = attention(hp, tt, get_qT(tt), ph, next_ctx, filler)
            for u in filler:
                u()
            if tt > 0:
                layer_norm(tt - 1)

        # tail: out_proj(3), one AllReduce (splitting it pays the ~15us
        # collective fixed cost twice, serialized -- measured worse), then
        # LN(3)
        last = NQT - 1
        for u in out_proj_units(last, get_qT(last), evac="scalar"):
            u()
        ar_unit(last)()
        layer_norm(last)

    persist.release()


_PROG = None


def _get_program():
    global _PROG
    if _PROG is None:
        _PROG = build_program()
    return _PROG


def _round_f32r(a):
    """Round fp32 to the f32r grid (11 explicit mantissa bits, RNE-ish)."""
    bits = np.ascontiguousarray(a, np.float32).view(np.uint32)
    r = ((bits.astype(np.uint64) + 0x800) & 0xFFFFF000).astype(np.uint32)
    return r.view(np.float32)


def make_in_maps(x, Wk, Wq, Wv, Wp, gamma, beta):
    import ml_dtypes
    x = np.asarray(x, dtype=np.float32)
    k = np.arange(KT)[:, None]
    q = np.arange(KT)[None, :]
    mask = (k <= q).astype(np.float32).astype(ml_dtypes.bfloat16)
    in_maps = []
    for c in range(8):
        b, hg = c // HG, c % HG
        sl = slice(hg * CG, (hg + 1) * CG)
        in_maps.append({
            "xT": _round_f32r(x[b].T),
            "wqT": _round_f32r(np.asarray(Wq, np.float32)[sl, :].T),
            "wkT": _round_f32r(np.asarray(Wk, np.float32)[sl, :].T),
            "wvT": _round_f32r(np.asarray(Wv, np.float32)[sl, :].T),
            "wpT": np.asarray(Wp, np.float32)[:, sl].T.astype(ml_dtypes.bfloat16),
            "gamma": np.asarray(gamma, np.float32).astype(ml_dtypes.bfloat16),
            "beta": np.asarray(beta, np.float32).astype(ml_dtypes.bfloat16),
            "mask": mask,
        })
    return in_maps


def kernel(x, Wk, Wq, Wv, Wp, gamma, beta, _trace=False, _trace_kwargs=None):
    nc, io = _get_program()
    in_maps = make_in_maps(x, Wk, Wq, Wv, Wp, gamma, beta)
    res = run_bass_kernel_spmd(
        nc, in_maps, core_ids=list(range(8)),
        trace=_trace, **(_trace_kwargs or {}),
    )
    out = np.stack([np.asarray(res.results[HG * b]["y"], dtype=np.float32)
                    for b in range(B)])
    if _trace:
        kernel.last_results = res
    return out
